# revision 13
# baseline (speedup 1.0000x reference)
"""Causal self-attention (B=4, T=2048, C=768, H=12) on 8 trn2 NeuronCores.

Sharding: core c -> batch b = c//2, head-half hh = c%2 (6 heads per core).
Each core computes, for its (b, 6 heads): qkv projection, causal attention,
and a partial output projection (its heads' rows of W_proj). The host sums
the two partial outputs per batch and adds b_proj.

All matmul operands are bf16 (fp32 accumulation in PSUM). Layouts keep the
PE contracting over partitions everywhere and softmax needs no transposes:
  - q^T, k^T [d, T]: weight-stationary qkv matmul, N=1024 moving chunks
  - S^T [tk, tq] blocks: lhsT = k^T tile, rhs = q^T chunk; two heads of a
    pair run concurrently via row groups (base partitions 0/64, K=64)
  - exp on ACT straight out of PSUM ([128, 1024] pair tiles, causal-skipped)
  - PV per block into one [128,1024] PSUM tile: cols 0:512 = [O_A; den_A]
    (rhs = P_A), cols 512:1024 = [den_B; O_B] (rhs = P_B); the ones block
    makes each head's softmax denominator come out replicated on the 64
    partitions opposite its O^T rows, so normalize = reciprocal +
    partition-swap DMA + elementwise multiply.
  - chunk-major schedule: for each 512-wide q chunk, attention for all 3
    head pairs then the output projection of that chunk's 4 t-tiles, so
    proj compute + output DMA overlap the remaining attention.
"""

import numpy as np

B, T, C = 4, 2048, 768
H = 12
D = C // H          # 64
HPC = 6             # heads per core
NP = 3              # head pairs per core
N_CORES = 8
TK = T // 128       # 16 t tiles
NCH = T // 512      # 4 q chunks
CT = C // 128       # 6 contraction tiles

_cache = {}


def _build(has_bias):
    import concourse.tile as tile
    from concourse import bacc, mybir

    dt = mybir.dt
    f32 = dt.float32
    bf16 = dt.bfloat16
    Exp = mybir.ActivationFunctionType.Exp

    nc = bacc.Bacc("TRN2", target_bir_lowering=False, debug=False,
                   num_devices=N_CORES)

    xT_ap = nc.dram_tensor("xT", [C, T], bf16, kind="ExternalInput").ap()
    wa_ap = nc.dram_tensor("wa", [C, 1152], bf16, kind="ExternalInput").ap()
    wp_ap = nc.dram_tensor("wp", [HPC * D, C], bf16, kind="ExternalInput").ap()
    tri_ap = nc.dram_tensor("tri", [128, 128], bf16, kind="ExternalInput").ap()
    if has_bias:
        ox_ap = nc.dram_tensor("ox", [1, T], bf16, kind="ExternalInput").ap()
        wb_ap = nc.dram_tensor("wb", [1, 1152], bf16, kind="ExternalInput").ap()
    out_ap = nc.dram_tensor("out", [T, C], f32, kind="ExternalOutput").ap()

    with tile.TileContext(nc) as tc:
        with tc.tile_pool(name="pers", bufs=1) as pers, \
             tc.tile_pool(name="pP", bufs=3) as pP, \
             tc.tile_pool(name="pst", bufs=2) as pst, \
             tc.tile_pool(name="pout", bufs=2) as pout, \
             tc.tile_pool(name="psA", bufs=2, space="PSUM") as psA, \
             tc.tile_pool(name="psOD", bufs=2, space="PSUM") as psOD:

            # ---- persistent SBUF tensors + staged input DMA ----
            # wa columns (host layout): q0 k0 q1 k1 q2 k2 (128 each) | v (384)
            xT = [pers.tile([128, T], bf16, tag=f"x{i}", name=f"x{i}")
                  for i in range(CT)]
            wa = [pers.tile([128, 1152], bf16, tag=f"w{i}", name=f"w{i}")
                  for i in range(CT)]
            # priority order: pair0 q/k weights, then x first half, x second
            # half, v weights, pair1/2 q/k weights.
            for i in range(CT):
                nc.sync.dma_start(wa[i][:, 0:256],
                                  wa_ap[i * 128:(i + 1) * 128, 0:256])
                nc.sync.dma_start(xT[i][:, 0:512],
                                  xT_ap[i * 128:(i + 1) * 128, 0:512])
            for q in range(1, 4):
                for i in range(CT):
                    nc.sync.dma_start(xT[i][:, q * 512:(q + 1) * 512],
                                      xT_ap[i * 128:(i + 1) * 128,
                                            q * 512:(q + 1) * 512])
            for i in range(CT):
                nc.sync.dma_start(wa[i][:, 768:1152],
                                  wa_ap[i * 128:(i + 1) * 128, 768:1152])
            for i in range(CT):
                nc.sync.dma_start(wa[i][:, 256:768],
                                  wa_ap[i * 128:(i + 1) * 128, 256:768])
            wp = [pers.tile([128, C], bf16, tag=f"p{i}", name=f"wp{i}")
                  for i in range(NP)]
            for i in range(NP):
                nc.sync.dma_start(wp[i][:], wp_ap[i * 128:(i + 1) * 128, :])
            tri_b = pers.tile([128, 128], bf16, tag="trib")
            nc.sync.dma_start(tri_b[:], tri_ap)
            if has_bias:
                ox = pers.tile([1, T], bf16, tag="ox")
                nc.sync.dma_start(ox[:], ox_ap)
                wb = pers.tile([1, 1152], bf16, tag="wb")
                nc.sync.dma_start(wb[:], wb_ap)
            ones_f = pers.tile([128, 64], f32, tag="onesf")
            nc.vector.memset(ones_f[:], 1.0)
            ones_b = pers.tile([128, 64], bf16, tag="onesb")
            nc.vector.tensor_copy(ones_b[:], ones_f[:])

            qk = [pers.tile([128, T], bf16, tag=f"qk{i}", name=f"qk{i}")
                  for i in range(2 * NP)]
            # v: one wide [128, 384] bf16 tile per t-tile (heads side by side)
            vv = [pers.tile([128, 384], bf16, tag=f"v{t}", name=f"v{t}")
                  for t in range(TK)]
            yt = [pers.tile([128, T], bf16, tag=f"y{p}", name=f"y{p}")
                  for p in range(NP)]

            # GPSIMD cannot access PSUM: every PSUM-touching elementwise op
            # stays on DVE; gpsimd gets the SBUF-only tri-masking.

            # ---- qkv projection pieces ----
            def emit_qk(p):
                with nc.named_scope(f"qk{p}"):
                    for qsel in range(2):          # 0 = q, 1 = k
                        dst = qk[2 * p + qsel]
                        wcol = p * 256 + qsel * 128
                        for cp in range(2):        # 1024-col output tiles
                            ps = psA.tile([128, 1024], f32, tag="A")
                            for half in range(2):  # PSUM-bank-sized matmuls
                                t0 = cp * 1024 + half * 512
                                for c in range(CT):
                                    nc.tensor.matmul(
                                        ps[:, half * 512:half * 512 + 512],
                                        lhsT=wa[c][:, wcol:wcol + 128],
                                        rhs=xT[c][:, t0:t0 + 512],
                                        start=(c == 0),
                                        stop=(c == CT - 1 and not has_bias))
                                if has_bias:
                                    nc.tensor.matmul(
                                        ps[:, half * 512:half * 512 + 512],
                                        lhsT=wb[0:1, wcol:wcol + 128],
                                        rhs=ox[0:1, t0:t0 + 512],
                                        start=False, stop=True)
                            nc.vector.tensor_copy(
                                dst[:, cp * 1024:(cp + 1) * 1024], ps[:])

            def emit_v(tp):                        # t-tile pair {2tp, 2tp+1}
                with nc.named_scope("qkv_v"):
                    ps = psA.tile([128, 1024], f32, tag="A")
                    for half in range(2):
                        t = 2 * tp + half
                        o = half * 512
                        for c in range(CT):
                            nc.tensor.matmul(
                                ps[:, o:o + 384],
                                lhsT=xT[c][:, t * 128:(t + 1) * 128],
                                rhs=wa[c][:, 768:1152],
                                start=(c == 0),
                                stop=(c == CT - 1 and not has_bias))
                        if has_bias:
                            nc.tensor.matmul(
                                ps[:, o:o + 384],
                                lhsT=ox[0:1, t * 128:(t + 1) * 128],
                                rhs=wb[0:1, 768:1152],
                                start=False, stop=True)
                    for half in range(2):
                        t = 2 * tp + half
                        nc.vector.tensor_copy(vv[t][:],
                                              ps[:, half * 512:half * 512 + 384])

            # ---- attention unit: pair p, q chunk j ----
            def emit_attn(p, j):
                qA = qk[2 * p]
                kA = qk[2 * p + 1]
                nblk = 4 * j + 4
                with nc.named_scope(f"attn{p}_{j}"):
                    pv = psOD.tile([128, 1024], f32, tag="OD")
                    Ps = [None] * nblk
                    ms = [None] * nblk

                    def emit_S(i):
                        m = i - 4 * j
                        lo = 128 * m if m >= 0 else 0
                        sp = psA.tile([128, 1024], f32, tag="A")
                        for ab in range(2):
                            nc.tensor.matmul(
                                sp[:, ab * 512 + lo:(ab + 1) * 512],
                                lhsT=kA[ab * 64:(ab + 1) * 64,
                                        i * 128:(i + 1) * 128],
                                rhs=qA[ab * 64:(ab + 1) * 64,
                                       j * 512 + lo:(j + 1) * 512],
                                start=True, stop=True)
                        P = pP.tile([128, 1024], bf16, tag="P")
                        if lo:
                            nc.scalar.activation(
                                P[:, lo:512], sp[:, lo:512], Exp)
                            nc.scalar.activation(
                                P[:, 512 + lo:1024], sp[:, 512 + lo:1024],
                                Exp)
                        else:
                            nc.scalar.activation(P[:], sp[:], Exp)
                        Ps[i], ms[i] = P, max(m, 0)

                    def emit_PV(i):
                        m = ms[i]
                        lo = 128 * m
                        P = Ps[i]
                        if m > 0 or i == 4 * j:
                            # diagonal sub-block masking (multiply by tri)
                            for ab in range(2):
                                sl = P[:, ab * 512 + lo:ab * 512 + lo + 128]
                                nc.gpsimd.tensor_mul(sl, sl, tri_b[:])
                        first, last = (i == 0), (i == nblk - 1)
                        vA = vv[i][:, (2 * p) * 64:(2 * p) * 64 + 64]
                        vB = vv[i][:, (2 * p + 1) * 64:(2 * p + 1) * 64 + 64]
                        # cols 0:512 from P_A -> [O_A; den_A],
                        # cols 512:1024 from P_B -> [den_B; O_B]
                        nc.tensor.matmul(
                            pv[0:64, lo:512], lhsT=vA,
                            rhs=P[:, lo:512], start=first, stop=last)
                        nc.tensor.matmul(
                            pv[64:128, lo:512], lhsT=ones_b[:],
                            rhs=P[:, lo:512], start=first, stop=last)
                        nc.tensor.matmul(
                            pv[0:64, 512 + lo:1024], lhsT=ones_b[:],
                            rhs=P[:, 512 + lo:1024], start=first, stop=last)
                        nc.tensor.matmul(
                            pv[64:128, 512 + lo:1024], lhsT=vB,
                            rhs=P[:, 512 + lo:1024], start=first, stop=last)

                    # software-pipeline: S(i+1) is emitted before PV(i)
                    emit_S(0)
                    for i in range(1, nblk):
                        emit_S(i)
                        emit_PV(i - 1)
                    emit_PV(nblk - 1)

                    # normalize. First evacuate the whole PV accumulator to
                    # SBUF in one copy so the PSUM tile frees immediately;
                    # the reciprocal dance + multiplies then run from SBUF
                    # (multiplies on the otherwise-idle gpsimd engine).
                    # reciprocal_approx_fast only works at base partition 0:
                    # den_B (rows 0-63 of the B half) is recip'd directly;
                    # den_A (rows 64-127 of the A half) is swapped down via
                    # DMA, recip'd, while recip(den_B) swaps up.
                    sb = pst.tile([128, 1024], f32, tag="sb")
                    nc.vector.tensor_copy(sb[:], pv[:])
                    rB = pst.tile([64, 512], f32, tag="rB")
                    nc.vector.reciprocal_approx_fast(
                        rB[:], sb[0:64, 512:1024])
                    s2 = pst.tile([128, 512], f32, tag="rc")
                    nc.sync.dma_start(s2[0:64, :], sb[64:128, 0:512])
                    nc.sync.dma_start(s2[64:128, :], rB[:])
                    s3 = pst.tile([64, 512], f32, tag="s3")
                    nc.vector.reciprocal_approx_fast(s3[:], s2[0:64, :])
                    nc.gpsimd.tensor_mul(
                        yt[p][0:64, j * 512:(j + 1) * 512],
                        sb[0:64, 0:512], s3[:])
                    nc.gpsimd.tensor_mul(
                        yt[p][64:128, j * 512:(j + 1) * 512],
                        sb[64:128, 512:1024], s2[64:128, :])

            # ---- output projection for chunk j (t-tiles 4j..4j+3) ----
            def emit_proj(j):
                with nc.named_scope(f"proj{j}"):
                    for t in range(4 * j, 4 * j + 4):
                        ps = psA.tile([128, 1024], f32, tag="A")
                        for n0, n1 in ((0, 512), (512, 768)):
                            for kk in range(NP):
                                nc.tensor.matmul(
                                    ps[:, n0:n1],
                                    lhsT=yt[kk][:, t * 128:(t + 1) * 128],
                                    rhs=wp[kk][:, n0:n1],
                                    start=(kk == 0), stop=(kk == NP - 1))
                        ob = pout.tile([128, C], f32, tag="o")
                        nc.vector.tensor_copy(ob[:], ps[:, 0:C])
                        nc.sync.dma_start(
                            out_ap[t * 128:(t + 1) * 128, :], ob[:])

            # ---- emission schedule ----
            emit_qk(0)
            emit_v(0)
            emit_v(1)
            emit_attn(0, 0)
            for tp in range(2, 8):
                emit_v(tp)
            emit_qk(1)
            emit_attn(1, 0)
            # proj(j) is emitted one chunk late so the next chunk's S tiles
            # get their PSUM slots (and the scalar engine its exp stream)
            # without waiting on normalize(j) -> proj(j).
            emit_qk(2)
            emit_attn(2, 0)
            for j in range(1, NCH):
                for p in range(NP):
                    emit_attn(p, j)
                emit_proj(j - 1)
            emit_proj(NCH - 1)

    nc.compile()
    return nc


def _prep_inputs(x, W_qkv, b_qkv, W_proj):
    """Per-core input maps (bf16 host arrays)."""
    import ml_dtypes
    bf = ml_dtypes.bfloat16
    sc = 1.0 / np.sqrt(D)
    tri = np.triu(np.ones((128, 128), dtype=np.float32)).astype(bf)
    in_maps = []
    for c in range(N_CORES):
        b, hh = c // 2, c % 2
        h0 = hh * 384                      # column offset of this half's heads
        # wa column order: q0 k0 q1 k1 q2 k2 (128 each) | v (384)
        pieces = []
        for p in range(NP):
            pieces.append(W_qkv[:, h0 + p * 128:h0 + (p + 1) * 128] * sc)
            pieces.append(W_qkv[:, 768 + h0 + p * 128:768 + h0 + (p + 1) * 128])
        pieces.append(W_qkv[:, 1536 + h0:1536 + h0 + 384])
        wa = np.ascontiguousarray(
            np.concatenate(pieces, axis=1)).astype(bf)
        m = {
            "xT": np.ascontiguousarray(x[b].T).astype(bf),
            "wa": wa,
            "wp": np.ascontiguousarray(W_proj[h0:h0 + 384, :]).astype(bf),
            "tri": tri,
        }
        if np.any(b_qkv):
            bp = []
            for p in range(NP):
                bp.append(b_qkv[h0 + p * 128:h0 + (p + 1) * 128] * sc)
                bp.append(b_qkv[768 + h0 + p * 128:768 + h0 + (p + 1) * 128])
            bp.append(b_qkv[1536 + h0:1536 + h0 + 384])
            m["ox"] = np.ones((1, T), dtype=np.float32).astype(bf)
            m["wb"] = np.concatenate(bp).reshape(1, 1152).astype(bf)
        in_maps.append(m)
    return in_maps


def _run(inputs, trace=False, tmpdir=None):
    from concourse.bass_utils import run_bass_kernel_spmd

    x = np.asarray(inputs["x"], dtype=np.float32)
    W_qkv = np.asarray(inputs["W_qkv"], dtype=np.float32)
    b_qkv = np.asarray(inputs["b_qkv"], dtype=np.float32)
    W_proj = np.asarray(inputs["W_proj"], dtype=np.float32)
    b_proj = np.asarray(inputs["b_proj"], dtype=np.float32)

    has_bias = bool(np.any(b_qkv))
    key = ("k", has_bias)
    if key not in _cache:
        _cache[key] = _build(has_bias)
    nc = _cache[key]

    in_maps = _prep_inputs(x, W_qkv, b_qkv, W_proj)
    res = run_bass_kernel_spmd(nc, in_maps, list(range(N_CORES)),
                               trace=trace, tmpdir=tmpdir)
    out = np.empty((B, T, C), dtype=np.float32)
    for b in range(B):
        out[b] = res.results[2 * b]["out"] + res.results[2 * b + 1]["out"]
    out += b_proj
    return out, res


def kernel(**inputs):
    out, _ = _run(inputs)
    return out


# revision 17
# speedup vs baseline: 1.0804x; 1.0804x over previous
"""Causal self-attention (B=4, T=2048, C=768, H=12) on 8 trn2 NeuronCores.

Sharding: core c -> batch b = c//2, head-half hh = c%2 (6 heads per core).
Each core computes, for its (b, 6 heads): qkv projection, causal attention,
and a partial output projection (its heads' rows of W_proj). The host sums
the two partial outputs per batch and adds b_proj.

All matmul operands are bf16 (fp32 accumulation in PSUM). Layouts keep the
PE contracting over partitions everywhere and softmax needs no transposes:
  - q^T, k^T [d, T]: weight-stationary qkv matmul, N=1024 moving chunks
  - S^T [tk, tq] blocks: lhsT = k^T tile, rhs = q^T chunk; two heads of a
    pair run concurrently via row groups (base partitions 0/64, K=64)
  - exp on ACT straight out of PSUM ([128, 1024] pair tiles, causal-skipped)
  - PV per block into one [128,1024] PSUM tile: cols 0:512 = [O_A; den_A]
    (rhs = P_A), cols 512:1024 = [den_B; O_B] (rhs = P_B); the ones block
    makes each head's softmax denominator come out replicated on the 64
    partitions opposite its O^T rows, so normalize = reciprocal +
    partition-swap DMA + elementwise multiply.
  - chunk-major schedule: for each 512-wide q chunk, attention for all 3
    head pairs then the output projection of that chunk's 4 t-tiles, so
    proj compute + output DMA overlap the remaining attention.
"""

import numpy as np

B, T, C = 4, 2048, 768
H = 12
D = C // H          # 64
HPC = 6             # heads per core
NP = 3              # head pairs per core
N_CORES = 8
TK = T // 128       # 16 t tiles
NCH = T // 512      # 4 q chunks
CT = C // 128       # 6 contraction tiles

_cache = {}


def _build(has_bias):
    import concourse.tile as tile
    from concourse import bacc, mybir

    dt = mybir.dt
    f32 = dt.float32
    bf16 = dt.bfloat16
    Exp = mybir.ActivationFunctionType.Exp

    nc = bacc.Bacc("TRN2", target_bir_lowering=False, debug=False,
                   num_devices=N_CORES)

    xT_ap = nc.dram_tensor("xT", [C, T], bf16, kind="ExternalInput").ap()
    wa_ap = nc.dram_tensor("wa", [C, 1152], bf16, kind="ExternalInput").ap()
    wp_ap = nc.dram_tensor("wp", [HPC * D, C], bf16, kind="ExternalInput").ap()
    tri_ap = nc.dram_tensor("tri", [128, 128], bf16, kind="ExternalInput").ap()
    if has_bias:
        ox_ap = nc.dram_tensor("ox", [1, T], bf16, kind="ExternalInput").ap()
        wb_ap = nc.dram_tensor("wb", [1, 1152], bf16, kind="ExternalInput").ap()
    out_ap = nc.dram_tensor("out", [T, C], f32, kind="ExternalOutput").ap()

    with tile.TileContext(nc) as tc:
        with tc.tile_pool(name="pers", bufs=1) as pers, \
             tc.tile_pool(name="pP", bufs=3) as pP, \
             tc.tile_pool(name="pst", bufs=2) as pst, \
             tc.tile_pool(name="pout", bufs=2) as pout, \
             tc.tile_pool(name="psA", bufs=2, space="PSUM") as psA, \
             tc.tile_pool(name="psOD", bufs=2, space="PSUM") as psOD:

            # ---- persistent SBUF tensors + staged input DMA ----
            # wa columns (host layout): q0 k0 q1 k1 q2 k2 (128 each) | v (384)
            xT = [pers.tile([128, T], bf16, tag=f"x{i}", name=f"x{i}")
                  for i in range(CT)]
            wa = [pers.tile([128, 1152], bf16, tag=f"w{i}", name=f"w{i}")
                  for i in range(CT)]
            # priority order: pair0 q/k weights, then x first half, x second
            # half, v weights, pair1/2 q/k weights.
            for i in range(CT):
                nc.sync.dma_start(wa[i][:, 0:256],
                                  wa_ap[i * 128:(i + 1) * 128, 0:256])
                nc.sync.dma_start(xT[i][:, 0:512],
                                  xT_ap[i * 128:(i + 1) * 128, 0:512])
            for q in range(1, 4):
                for i in range(CT):
                    nc.sync.dma_start(xT[i][:, q * 512:(q + 1) * 512],
                                      xT_ap[i * 128:(i + 1) * 128,
                                            q * 512:(q + 1) * 512])
            for i in range(CT):
                nc.sync.dma_start(wa[i][:, 768:1152],
                                  wa_ap[i * 128:(i + 1) * 128, 768:1152])
            for i in range(CT):
                nc.sync.dma_start(wa[i][:, 256:768],
                                  wa_ap[i * 128:(i + 1) * 128, 256:768])
            wp = [pers.tile([128, C], bf16, tag=f"p{i}", name=f"wp{i}")
                  for i in range(NP)]
            for i in range(NP):
                nc.sync.dma_start(wp[i][:], wp_ap[i * 128:(i + 1) * 128, :])
            tri_b = pers.tile([128, 128], bf16, tag="trib")
            nc.sync.dma_start(tri_b[:], tri_ap)
            if has_bias:
                ox = pers.tile([1, T], bf16, tag="ox")
                nc.sync.dma_start(ox[:], ox_ap)
                wb = pers.tile([1, 1152], bf16, tag="wb")
                nc.sync.dma_start(wb[:], wb_ap)
            ones_f = pers.tile([128, 64], f32, tag="onesf")
            nc.vector.memset(ones_f[:], 1.0)
            ones_b = pers.tile([128, 64], bf16, tag="onesb")
            nc.vector.tensor_copy(ones_b[:], ones_f[:])

            qk = [pers.tile([128, T], bf16, tag=f"qk{i}", name=f"qk{i}")
                  for i in range(2 * NP)]
            # v: one wide [128, 384] bf16 tile per t-tile (heads side by side)
            vv = [pers.tile([128, 384], bf16, tag=f"v{t}", name=f"v{t}")
                  for t in range(TK)]
            yt = [pers.tile([128, T], bf16, tag=f"y{p}", name=f"y{p}")
                  for p in range(NP)]

            # GPSIMD cannot access PSUM: every PSUM-touching elementwise op
            # stays on DVE; gpsimd gets the SBUF-only tri-masking.

            # ---- qkv projection pieces ----
            def emit_qk(p):
                with nc.named_scope(f"qk{p}"):
                    for qsel in range(2):          # 0 = q, 1 = k
                        dst = qk[2 * p + qsel]
                        wcol = p * 256 + qsel * 128
                        for cp in range(2):        # 1024-col output tiles
                            ps = psA.tile([128, 1024], f32, tag="A")
                            for half in range(2):  # PSUM-bank-sized matmuls
                                t0 = cp * 1024 + half * 512
                                for c in range(CT):
                                    nc.tensor.matmul(
                                        ps[:, half * 512:half * 512 + 512],
                                        lhsT=wa[c][:, wcol:wcol + 128],
                                        rhs=xT[c][:, t0:t0 + 512],
                                        start=(c == 0),
                                        stop=(c == CT - 1 and not has_bias))
                                if has_bias:
                                    nc.tensor.matmul(
                                        ps[:, half * 512:half * 512 + 512],
                                        lhsT=wb[0:1, wcol:wcol + 128],
                                        rhs=ox[0:1, t0:t0 + 512],
                                        start=False, stop=True)
                            nc.vector.tensor_copy(
                                dst[:, cp * 1024:(cp + 1) * 1024], ps[:])

            def emit_v(tp):                        # t-tile pair {2tp, 2tp+1}
                with nc.named_scope("qkv_v"):
                    ps = psA.tile([128, 1024], f32, tag="A")
                    for half in range(2):
                        t = 2 * tp + half
                        o = half * 512
                        for c in range(CT):
                            nc.tensor.matmul(
                                ps[:, o:o + 384],
                                lhsT=xT[c][:, t * 128:(t + 1) * 128],
                                rhs=wa[c][:, 768:1152],
                                start=(c == 0),
                                stop=(c == CT - 1 and not has_bias))
                        if has_bias:
                            nc.tensor.matmul(
                                ps[:, o:o + 384],
                                lhsT=ox[0:1, t * 128:(t + 1) * 128],
                                rhs=wb[0:1, 768:1152],
                                start=False, stop=True)
                    for half in range(2):
                        t = 2 * tp + half
                        nc.vector.tensor_copy(vv[t][:],
                                              ps[:, half * 512:half * 512 + 384])

            # ---- attention unit: pair p, q chunk j ----
            def emit_attn(p, j):
                qA = qk[2 * p]
                kA = qk[2 * p + 1]
                nblk = 4 * j + 4
                with nc.named_scope(f"attn{p}_{j}"):
                    pv = psOD.tile([128, 1024], f32, tag="OD")
                    Ps = [None] * nblk
                    ms = [None] * nblk

                    def emit_S(i):
                        m = i - 4 * j
                        lo = 128 * m if m >= 0 else 0
                        sp = psA.tile([128, 1024], f32, tag="A")
                        for ab in range(2):
                            nc.tensor.matmul(
                                sp[:, ab * 512 + lo:(ab + 1) * 512],
                                lhsT=kA[ab * 64:(ab + 1) * 64,
                                        i * 128:(i + 1) * 128],
                                rhs=qA[ab * 64:(ab + 1) * 64,
                                       j * 512 + lo:(j + 1) * 512],
                                start=True, stop=True)
                        P = pP.tile([128, 1024], bf16, tag="P")
                        if lo:
                            # one ACT instruction for both heads' causal
                            # strips via a strided 3D access pattern
                            sp3 = sp[:].rearrange(
                                "p (h w) -> p h w", h=2)[:, :, lo:512]
                            P3 = P[:].rearrange(
                                "p (h w) -> p h w", h=2)[:, :, lo:512]
                            nc.scalar.activation(P3, sp3, Exp)
                        else:
                            nc.scalar.activation(P[:], sp[:], Exp)
                        Ps[i], ms[i] = P, max(m, 0)

                    def emit_PV(i):
                        m = ms[i]
                        lo = 128 * m
                        P = Ps[i]
                        if m > 0 or i == 4 * j:
                            # diagonal sub-block masking (multiply by tri),
                            # both heads' strips in one gpsimd op
                            sl = P[:].rearrange(
                                "p (h w) -> p h w", h=2)[:, :, lo:lo + 128]
                            tri3 = tri_b[:].unsqueeze(1).broadcast_to(
                                [128, 2, 128])
                            nc.gpsimd.tensor_mul(sl, sl, tri3)
                        first, last = (i == 0), (i == nblk - 1)
                        vA = vv[i][:, (2 * p) * 64:(2 * p) * 64 + 64]
                        vB = vv[i][:, (2 * p + 1) * 64:(2 * p + 1) * 64 + 64]
                        # cols 0:512 from P_A -> [O_A; den_A],
                        # cols 512:1024 from P_B -> [den_B; O_B]
                        nc.tensor.matmul(
                            pv[0:64, lo:512], lhsT=vA,
                            rhs=P[:, lo:512], start=first, stop=last)
                        nc.tensor.matmul(
                            pv[64:128, lo:512], lhsT=ones_b[:],
                            rhs=P[:, lo:512], start=first, stop=last)
                        nc.tensor.matmul(
                            pv[0:64, 512 + lo:1024], lhsT=ones_b[:],
                            rhs=P[:, 512 + lo:1024], start=first, stop=last)
                        nc.tensor.matmul(
                            pv[64:128, 512 + lo:1024], lhsT=vB,
                            rhs=P[:, 512 + lo:1024], start=first, stop=last)

                    # software-pipeline: S(i+1) is emitted before PV(i)
                    emit_S(0)
                    for i in range(1, nblk):
                        emit_S(i)
                        emit_PV(i - 1)
                    emit_PV(nblk - 1)

                    # normalize. First evacuate the whole PV accumulator to
                    # SBUF in one copy so the PSUM tile frees immediately;
                    # the reciprocal dance + multiplies then run from SBUF
                    # (multiplies on the otherwise-idle gpsimd engine).
                    # reciprocal_approx_fast only works at base partition 0:
                    # den_B (rows 0-63 of the B half) is recip'd directly;
                    # den_A (rows 64-127 of the A half) is swapped down via
                    # DMA, recip'd, while recip(den_B) swaps up.
                    sb = pst.tile([128, 1024], f32, tag="sb")
                    nc.vector.tensor_copy(sb[:], pv[:])
                    rB = pst.tile([64, 512], f32, tag="rB")
                    nc.vector.reciprocal_approx_fast(
                        rB[:], sb[0:64, 512:1024])
                    s2 = pst.tile([128, 512], f32, tag="rc")
                    nc.sync.dma_start(s2[0:64, :], sb[64:128, 0:512])
                    nc.sync.dma_start(s2[64:128, :], rB[:])
                    s3 = pst.tile([64, 512], f32, tag="s3")
                    nc.vector.reciprocal_approx_fast(s3[:], s2[0:64, :])
                    nc.gpsimd.tensor_mul(
                        yt[p][0:64, j * 512:(j + 1) * 512],
                        sb[0:64, 0:512], s3[:])
                    nc.gpsimd.tensor_mul(
                        yt[p][64:128, j * 512:(j + 1) * 512],
                        sb[64:128, 512:1024], s2[64:128, :])

            # ---- output projection of one t-tile ----
            def emit_proj_t(t):
                with nc.named_scope("proj"):
                    ps = psA.tile([128, 1024], f32, tag="A")
                    for n0, n1 in ((0, 512), (512, 768)):
                        for kk in range(NP):
                            nc.tensor.matmul(
                                ps[:, n0:n1],
                                lhsT=yt[kk][:, t * 128:(t + 1) * 128],
                                rhs=wp[kk][:, n0:n1],
                                start=(kk == 0), stop=(kk == NP - 1))
                    ob = pout.tile([128, C], f32, tag="o")
                    nc.vector.tensor_copy(ob[:], ps[:, 0:C])
                    nc.sync.dma_start(
                        out_ap[t * 128:(t + 1) * 128, :], ob[:])

            # ---- emission schedule ----
            emit_qk(0)
            emit_v(0)
            emit_v(1)
            emit_attn(0, 0)
            for tp in range(2, 8):
                emit_v(tp)
            emit_qk(1)
            emit_attn(1, 0)
            # proj of chunk j is emitted one chunk late, one t-tile after
            # each attention unit of chunk j+1, so the PE absorbs proj work
            # in the exp-paced slack without stalling the next chunk's S
            # tiles (and the scalar engine's exp stream) behind it.
            emit_qk(2)
            emit_attn(2, 0)
            for j in range(1, NCH):
                t0 = 4 * (j - 1)
                emit_attn(0, j)
                emit_proj_t(t0)
                emit_attn(1, j)
                emit_proj_t(t0 + 1)
                emit_attn(2, j)
                emit_proj_t(t0 + 2)
                emit_proj_t(t0 + 3)
            for t in range(4 * (NCH - 1), T // 128):
                emit_proj_t(t)

    nc.compile()
    return nc


def _prep_inputs(x, W_qkv, b_qkv, W_proj):
    """Per-core input maps (bf16 host arrays)."""
    import ml_dtypes
    bf = ml_dtypes.bfloat16
    sc = 1.0 / np.sqrt(D)
    tri = np.triu(np.ones((128, 128), dtype=np.float32)).astype(bf)
    in_maps = []
    for c in range(N_CORES):
        b, hh = c // 2, c % 2
        h0 = hh * 384                      # column offset of this half's heads
        # wa column order: q0 k0 q1 k1 q2 k2 (128 each) | v (384)
        pieces = []
        for p in range(NP):
            pieces.append(W_qkv[:, h0 + p * 128:h0 + (p + 1) * 128] * sc)
            pieces.append(W_qkv[:, 768 + h0 + p * 128:768 + h0 + (p + 1) * 128])
        pieces.append(W_qkv[:, 1536 + h0:1536 + h0 + 384])
        wa = np.ascontiguousarray(
            np.concatenate(pieces, axis=1)).astype(bf)
        m = {
            "xT": np.ascontiguousarray(x[b].T).astype(bf),
            "wa": wa,
            "wp": np.ascontiguousarray(W_proj[h0:h0 + 384, :]).astype(bf),
            "tri": tri,
        }
        if np.any(b_qkv):
            bp = []
            for p in range(NP):
                bp.append(b_qkv[h0 + p * 128:h0 + (p + 1) * 128] * sc)
                bp.append(b_qkv[768 + h0 + p * 128:768 + h0 + (p + 1) * 128])
            bp.append(b_qkv[1536 + h0:1536 + h0 + 384])
            m["ox"] = np.ones((1, T), dtype=np.float32).astype(bf)
            m["wb"] = np.concatenate(bp).reshape(1, 1152).astype(bf)
        in_maps.append(m)
    return in_maps


def _run(inputs, trace=False, tmpdir=None):
    from concourse.bass_utils import run_bass_kernel_spmd

    x = np.asarray(inputs["x"], dtype=np.float32)
    W_qkv = np.asarray(inputs["W_qkv"], dtype=np.float32)
    b_qkv = np.asarray(inputs["b_qkv"], dtype=np.float32)
    W_proj = np.asarray(inputs["W_proj"], dtype=np.float32)
    b_proj = np.asarray(inputs["b_proj"], dtype=np.float32)

    has_bias = bool(np.any(b_qkv))
    key = ("k", has_bias)
    if key not in _cache:
        _cache[key] = _build(has_bias)
    nc = _cache[key]

    in_maps = _prep_inputs(x, W_qkv, b_qkv, W_proj)
    res = run_bass_kernel_spmd(nc, in_maps, list(range(N_CORES)),
                               trace=trace, tmpdir=tmpdir)
    out = np.empty((B, T, C), dtype=np.float32)
    for b in range(B):
        out[b] = res.results[2 * b]["out"] + res.results[2 * b + 1]["out"]
    out += b_proj
    return out, res


def kernel(**inputs):
    out, _ = _run(inputs)
    return out
